# revision 16
# baseline (speedup 1.0000x reference)
"""BottleneckAttention TRN2 kernel: 8 NeuronCores, one (batch, head) pair per core.

Decomposition (per core, batch b / head i):
  q = (scale * Wq_i) @ x_b          [64, 4096]   (d-major)
  k = Wk_i @ x_b                    [64, 4096]
  vT = (Wv_i @ x_b)^T               [4096, 64]   (n-major, built chunkwise)
  Height rel-bias folded into the score matmul via an augmented contraction:
     K_aug = [k; Ih]  Q_aug = [q; RH^T]     (Ih[h',j] = 1 if j//64 == h')
     S^T[j,q] = K_aug^T Q_aug = content + height-bias
  Width rel-bias applied multiplicatively after exp (separability of exp):
     E = exp(S^T) * ew_dup[jw(j), q],  ew = exp(RW^T)
  PV + row-sums fused: vT_aug = [vT | 1] so out rows 0..63 = unnormalized
  attention output (transposed [d, q]), row 64 = softmax denominators.
  Output projection partial: P = Wout[:, i*64:(i+1)*64] @ out, then columns
  scaled by 1/sums (normalization commutes with the d-contraction).
Host sums the 4 per-head partials per batch and adds the residual x.

All inputs are pre-converted to bf16 on host (no on-device casts). The
steady-state pipeline is ACT(exp)-bound at ~1.1us per 128-key chunk; the
schedule keeps ACT saturated: PE builds/projections go to a dedicated PSUM
slot, psum->sbuf copies go to DVE, softmax denominators use the fast
approximate reciprocal, ew is exp'd in per-quarter chunks so quarter 0 can
start before the full width table is ready.
"""

import numpy as np
import ml_dtypes

import concourse.bass as bass
import concourse.bacc as bacc
import concourse.tile as tile
from concourse import mybir
from concourse.bass_utils import run_bass_kernel_spmd

F32 = mybir.dt.float32
BF16 = mybir.dt.bfloat16
AF = mybir.ActivationFunctionType

HEADS, B, C, HH, WW = 4, 2, 256, 64, 64
N = HH * WW           # 4096
DH = C // HEADS       # 64
NQ = 4                # query blocks
QB = N // NQ          # 1024 query cols per block
NJC = 32              # key chunks of 128
PVLAG = 5


def _body(tc, io):
    from contextlib import ExitStack
    with ExitStack() as ctx:
        _body_inner(tc, io, ctx)


def _body_inner(tc, io, ctx):
    nc = tc.nc
    xb, wq, wk, wv, wo, relw, relh, ih, out = (
        io["xb"], io["wq"], io["wk"], io["wv"], io["wo"],
        io["relw"], io["relh"], io["ih"], io["out"],
    )

    big = ctx.enter_context(tc.tile_pool(name="big", bufs=1))
    rot = ctx.enter_context(tc.tile_pool(name="rot", bufs=10))
    ep = ctx.enter_context(tc.tile_pool(name="ep", bufs=2))
    spool = ctx.enter_context(tc.tile_pool(name="spool", bufs=2, space="PSUM"))
    opool = ctx.enter_context(tc.tile_pool(name="opool", bufs=1, space="PSUM"))
    ipool = ctx.enter_context(tc.tile_pool(name="ipool", bufs=1, space="PSUM"))
    dpool = ctx.enter_context(tc.tile_pool(name="dpool", bufs=2, space="DRAM"))

    # ---- SBUF tiles -------------------------------------------------
    xb_bf = big.tile([128, 2, N], BF16)
    wq_bf = big.tile([128, 2, DH], BF16)
    wk_bf = big.tile([128, 2, DH], BF16)
    wv_bf = big.tile([128, 2, DH], BF16)
    wo_bf = big.tile([64, 256], BF16)
    relw_bf = big.tile([64, 127], BF16)
    relh_bf = big.tile([64, 127], BF16)
    K_aug = big.tile([128, N], BF16)
    Q_aug = big.tile([128, N], BF16)
    ew_dup = big.tile([128, N], BF16)
    rwt = big.tile([64, N], BF16)
    vt_aug = big.tile([128, NJC, 65], BF16)
    h_sb = big.tile([64, N], BF16)

    # ---- input DMAs ------------------------------------------------
    # First-needed data first; weight loads go on the gpsimd queue so the
    # sync queue's issue cost doesn't delay xb quarter 0.
    xv = xb.rearrange("(cc p) n -> p cc n", p=128)
    nc.sync.dma_start(out=xb_bf[:, 0, 0:QB], in_=xv[:, 0, 0:QB])
    nc.gpsimd.dma_start(out=xb_bf[:, 1, 0:QB], in_=xv[:, 1, 0:QB])
    for t_bf, t_d in ((wq_bf, wq), (wk_bf, wk), (wv_bf, wv)):
        nc.gpsimd.dma_start(out=t_bf, in_=t_d.rearrange("(cc p) d -> p cc d", p=128))
    nc.sync.dma_start(out=relh_bf, in_=relh)
    for qq in range(1, NQ):
        nc.sync.dma_start(out=xb_bf[:, 0, bass.ts(qq, QB)], in_=xv[:, 0, bass.ts(qq, QB)])
        nc.gpsimd.dma_start(out=xb_bf[:, 1, bass.ts(qq, QB)], in_=xv[:, 1, bass.ts(qq, QB)])
    nc.sync.dma_start(out=relw_bf, in_=relw)
    # Ih rows of K_aug straight from dram (bf16, exact 0/1)
    nc.sync.dma_start(out=K_aug[64:128, :], in_=ih)
    nc.gpsimd.dma_start(out=wo_bf, in_=wo)

    nc.gpsimd.memset(vt_aug[:, :, 64:65], 1.0)

    # PE warm-up while the first xb quarter lands.
    warm = big.tile([128, 512], BF16)
    nc.vector.memset(warm, 0.0)
    for _ in range(9):
        wps = spool.tile([128, 512], F32, tag="sp")
        nc.tensor.matmul(wps, warm[:, 0:128], warm, start=True, stop=True)

    # ---- build helpers ---------------------------------------------
    # psum->sbuf copies: ACT while it is idle (prologue), DVE in-loop.
    def _copy(eng, dst, src):
        if eng == "act":
            nc.scalar.activation(out=dst, in_=src, func=AF.Copy)
        else:
            nc.vector.tensor_copy(out=dst, in_=src)

    def qk_build(dst, w_bf, qq, pool, tag, eng="dve"):
        ps = pool.tile([128, QB], F32, tag=tag)
        for cc in range(2):
            for h in range(2):
                nc.tensor.matmul(
                    ps[0:64, bass.ts(h, 512)],
                    w_bf[:, cc, :],
                    xb_bf[:, cc, qq * QB + h * 512: qq * QB + (h + 1) * 512],
                    start=(cc == 0), stop=(cc == 1),
                )
        _copy(eng, dst[0:64, bass.ts(qq, QB)], ps[0:64, :])

    def rh_build(g, pool, tag, eng="dve"):
        # RH^T[jh, n=(x,y)] = sum_d relh[jh - x + 63, d] * q[d, n]
        ps = pool.tile([128, QB], F32, tag=tag)
        for xi in range(16):
            xx = g * 16 + xi
            nc.tensor.matmul(
                ps[0:64, bass.ts(xi, 64)],
                relh_bf[:, 63 - xx: 127 - xx],
                Q_aug[0:64, xx * 64: (xx + 1) * 64],
                start=True, stop=True,
            )
        _copy(eng, Q_aug[64:128, bass.ts(g, QB)], ps[0:64, :])

    q_xy = Q_aug[0:64, :].rearrange("d (x y) -> d x y", y=64)
    rwt_xy = rwt.rearrange("jw (x y) -> jw x y", y=64)

    def rwx_build(qq, pool, tag, eng="dve"):
        # RW^T[jw, n=(x,y)] = sum_d relw[jw - y + 63, d] * q[d, n]
        # Built per query x-block so it only needs Q quarter qq (the serial
        # prologue chain xb -> Q -> rw -> ew -> e-muls stays per-quarter).
        ps = pool.tile([128, QB], F32, tag=tag)
        for y in range(64):
            nc.tensor.matmul(
                ps[0:64, y * 16:(y + 1) * 16],
                relw_bf[:, 63 - y: 127 - y],
                q_xy[:, qq * 16:(qq + 1) * 16, y],
                start=True, stop=True,
            )
        # ps free layout is [y, x]; rwt quarter slice wants [x, y].
        _copy(eng, rwt_xy[:, qq * 16:(qq + 1) * 16, :],
              ps[0:64, :].rearrange("p (y x) -> p x y", x=16))

    def vt_build(g, pool, tag, eng="dve"):
        ps = pool.tile([128, 8, 64], F32, tag=tag)
        for ci in range(8):
            chunk = g * 8 + ci
            for cc in range(2):
                nc.tensor.matmul(
                    ps[:, ci, :],
                    xb_bf[:, cc, chunk * 128: (chunk + 1) * 128],
                    wv_bf[:, cc, :],
                    start=(cc == 0), stop=(cc == 1),
                )
        _copy(eng, vt_aug[:, g * 8: (g + 1) * 8, 0:64], ps)

    def ew_build(c):
        # ew chunk c covers query x-block c; only quarter c's muls need it.
        nc.scalar.activation(out=ew_dup[0:64, bass.ts(c, QB)],
                             in_=rwt[:, bass.ts(c, QB)], func=AF.Exp)
        nc.vector.tensor_copy(out=ew_dup[64:128, bass.ts(c, QB)],
                              in_=ew_dup[0:64, bass.ts(c, QB)])

    # ---- prologue --------------------------------------------------
    # Only quarter 0's gates: Q0/rh0 and K0/V0 feed S(0); rwx0 -> ew0 feeds
    # the first e-mul. Everything else is injected into the loop.
    # Copy engines: ACT takes the chain that gates its own exp stream
    # (Q0/Q1/rh0), DVE takes the rest so the two chains run in parallel.
    qk_build(Q_aug, wq_bf, 0, spool, "sp", eng="act")
    qk_build(Q_aug, wq_bf, 1, spool, "sp", eng="act")
    rh_build(0, spool, "sp", eng="act")
    qk_build(K_aug, wk_bf, 0, spool, "sp", eng="dve")
    vt_build(0, spool, "sp", eng="dve")
    rwx_build(0, spool, "sp", eng="dve")
    ew_build(0)

    # ---- main attention loop ---------------------------------------
    # Per chunk: S^T matmul (PE) -> exp (ACT) -> *ew (DVE) -> PV (PE).
    # ACT is the bottleneck engine; everything else is scheduled around it.
    def make_proj(qqp, rbc):
        def proj(oh, pool=ipool, tag="ij"):
            pp = pool.tile([128, QB], F32, tag=tag)
            for h in range(2):
                nc.tensor.matmul(
                    pp[:, bass.ts(h, 512)],
                    wo_bf[:, oh * 128: (oh + 1) * 128],
                    h_sb[:, qqp * QB + h * 512: qqp * QB + (h + 1) * 512],
                    start=True, stop=True,
                )
            osb = ep.tile([128, QB], F32, tag="osb")
            nc.vector.tensor_mul(osb, pp, rbc)
            eng = nc.sync if oh == 0 else nc.gpsimd
            eng.dma_start(
                out=out[oh * 128: (oh + 1) * 128, qqp * QB: (qqp + 1) * QB],
                in_=osb,
            )
        return proj

    o_ps = None
    proj_prev = None

    def drain(qqp):
        # softmax denominators -> 1/sums broadcast to 128 partitions via a
        # bf16 dram round-trip; unnormalized attention rows -> h_sb (bf16).
        # Order: recip chain first so the DMA round-trip overlaps the h cast.
        rs0 = ep.tile([1, QB], F32, tag="rs0")
        nc.vector.tensor_copy(out=rs0, in_=o_ps[64:65, :])
        rsb = ep.tile([1, QB], F32, tag="rsb")
        nc.vector.reciprocal_approx_fast(out=rsb, in_=rs0)
        rsb_bf = ep.tile([1, QB], BF16, tag="rsbf")
        nc.vector.tensor_copy(out=rsb_bf, in_=rsb)
        rdram = dpool.tile([1, QB], BF16, tag="rd")
        nc.sync.dma_start(out=rdram, in_=rsb_bf)
        rbc = ep.tile([128, QB], BF16, tag="rbc")
        nc.sync.dma_start(
            out=rbc,
            in_=bass.AP(tensor=rdram.tensor, offset=rdram.offset,
                        ap=[[0, 128]] + list(rdram.ap[1:])),
        )
        nc.vector.tensor_copy(out=h_sb[:, bass.ts(qqp, QB)], in_=o_ps[0:64, :])
        return make_proj(qqp, rbc)

    for qq in range(NQ):
        if qq > 0:
            proj_prev = drain(qq - 1)
        o_ps = opool.tile([128, QB], F32, tag="ov")
        e_tiles = [None] * NJC

        def s_stage(jc):
            ps = spool.tile([128, QB], F32, tag="sp")
            for h in range(2):
                nc.tensor.matmul(
                    ps[:, bass.ts(h, 512)],
                    K_aug[:, jc * 128: (jc + 1) * 128],
                    Q_aug[:, qq * QB + h * 512: qq * QB + (h + 1) * 512],
                    start=True, stop=True,
                )
            e0 = rot.tile([128, QB], BF16, tag="e0")
            nc.scalar.activation(out=e0, in_=ps, func=AF.Exp)
            e = rot.tile([128, QB], BF16, tag="e")
            nc.vector.tensor_mul(e, e0, ew_dup[:, bass.ts(qq, QB)])
            e_tiles[jc] = e

        def pv_stage(jc):
            for h in range(2):
                nc.tensor.matmul(
                    o_ps[0:65, bass.ts(h, 512)],
                    vt_aug[:, jc, :],
                    e_tiles[jc][:, bass.ts(h, 512)],
                    start=(jc == 0), stop=(jc == NJC - 1),
                )
            e_tiles[jc] = None

        for t in range(NJC + PVLAG):
            if t < NJC:
                s_stage(t)
            if qq == 0:
                if t == 2:
                    qk_build(Q_aug, wq_bf, 2, ipool, "ij")
                elif t == 4:
                    qk_build(K_aug, wk_bf, 1, ipool, "ij")
                elif t == 6:
                    vt_build(1, ipool, "ij")
                elif t == 8:
                    rwx_build(1, ipool, "ij")
                elif t == 10:
                    qk_build(K_aug, wk_bf, 2, ipool, "ij")
                elif t == 12:
                    rh_build(1, ipool, "ij")
                elif t == 14:
                    vt_build(2, ipool, "ij")
                elif t == 16:
                    qk_build(Q_aug, wq_bf, 3, ipool, "ij")
                elif t == 18:
                    qk_build(K_aug, wk_bf, 3, ipool, "ij")
                elif t == 20:
                    vt_build(3, ipool, "ij")
                elif t == 24:
                    ew_build(1)
            else:
                if t == 6:
                    proj_prev(0)
                elif t == 10:
                    proj_prev(1)
                elif qq == 1:
                    if t == 2:
                        rwx_build(2, ipool, "ij")
                    elif t == 4:
                        rh_build(2, ipool, "ij")
                    elif t == 20:
                        rwx_build(3, ipool, "ij")
                    elif t == 24:
                        ew_build(2)
                elif qq == 2:
                    if t == 2:
                        rh_build(3, ipool, "ij")
                    elif t == 24:
                        ew_build(3)
            if t >= PVLAG:
                pv_stage(t - PVLAG)

    # final quarter epilogue: projections on free S-pool slots so the two
    # output halves run in parallel instead of serializing through ipool.
    proj_last = drain(NQ - 1)
    proj_last(0, pool=spool, tag="sp")
    proj_last(1, pool=spool, tag="sp")


_NC_CACHE = {}


def _build():
    if "nc" in _NC_CACHE:
        return _NC_CACHE["nc"]
    nc = bacc.Bacc("TRN2", target_bir_lowering=False, debug=False, num_devices=8)
    io = {
        "xb": nc.dram_tensor("xb", [C, N], BF16, kind="ExternalInput").ap(),
        "wq": nc.dram_tensor("wq", [C, DH], BF16, kind="ExternalInput").ap(),
        "wk": nc.dram_tensor("wk", [C, DH], BF16, kind="ExternalInput").ap(),
        "wv": nc.dram_tensor("wv", [C, DH], BF16, kind="ExternalInput").ap(),
        "wo": nc.dram_tensor("wo", [DH, C], BF16, kind="ExternalInput").ap(),
        "relw": nc.dram_tensor("relw", [DH, 127], BF16, kind="ExternalInput").ap(),
        "relh": nc.dram_tensor("relh", [DH, 127], BF16, kind="ExternalInput").ap(),
        "ih": nc.dram_tensor("ih", [64, N], BF16, kind="ExternalInput").ap(),
        "out": nc.dram_tensor("out", [C, N], F32, kind="ExternalOutput").ap(),
    }
    with tile.TileContext(nc) as tc:
        _body(tc, io)
    nc.compile()
    _NC_CACHE["nc"] = nc
    return nc


_last_in_maps = None


def kernel(x, w_qkv, w_out, rel_height, rel_width):
    global _last_in_maps
    bf16 = ml_dtypes.bfloat16
    x = np.ascontiguousarray(np.asarray(x, np.float32))
    w_qkv = np.asarray(w_qkv, np.float32)
    w_out = np.asarray(w_out, np.float32)
    rel_height = np.asarray(rel_height, np.float32)
    rel_width = np.asarray(rel_width, np.float32)

    scale = np.float32(DH ** -0.5)
    ih_const = np.ascontiguousarray(
        np.repeat(np.eye(64, dtype=np.float32), 64, axis=1).astype(bf16))
    relw_t = np.ascontiguousarray(rel_width.T.astype(bf16))
    relh_t = np.ascontiguousarray(rel_height.T.astype(bf16))

    xb_bf = [np.ascontiguousarray(x[b].reshape(C, N).astype(bf16)) for b in range(B)]

    in_maps = []
    for g in range(8):
        b, i = divmod(g, HEADS)
        sl = slice(i * DH, (i + 1) * DH)
        in_maps.append({
            "xb": xb_bf[b],
            "wq": np.ascontiguousarray((w_qkv[i * DH:(i + 1) * DH] * scale).T.astype(bf16)),
            "wk": np.ascontiguousarray(w_qkv[C + i * DH: C + (i + 1) * DH].T.astype(bf16)),
            "wv": np.ascontiguousarray(w_qkv[2 * C + i * DH: 2 * C + (i + 1) * DH].T.astype(bf16)),
            "wo": np.ascontiguousarray(w_out[:, sl].T.astype(bf16)),
            "relw": relw_t,
            "relh": relh_t,
            "ih": ih_const,
        })

    _last_in_maps = in_maps
    nc = _build()
    res = run_bass_kernel_spmd(nc, in_maps, core_ids=list(range(8)))
    parts = [r["out"] for r in res.results]
    outf = np.empty((B, C, N), np.float32)
    for b in range(B):
        outf[b] = parts[4 * b] + parts[4 * b + 1] + parts[4 * b + 2] + parts[4 * b + 3]
        outf[b] += x[b].reshape(C, N)
    return outf.reshape(B, C, HH, WW)


# revision 26
# speedup vs baseline: 1.0177x; 1.0177x over previous
"""BottleneckAttention TRN2 kernel: 8 NeuronCores, one (batch, head) pair per core.

Decomposition (per core, batch b / head i):
  q = (scale * Wq_i) @ x_b          [64, 4096]   (d-major)
  k = Wk_i @ x_b                    [64, 4096]
  vT = (Wv_i @ x_b)^T               [4096, 64]   (n-major, built chunkwise)
  Height rel-bias folded into the score matmul via an augmented contraction:
     K_aug = [k; Ih]  Q_aug = [q; RH^T]     (Ih[h',j] = 1 if j//64 == h')
     S^T[j,q] = K_aug^T Q_aug = content + height-bias
  Width rel-bias applied multiplicatively after exp (separability of exp):
     E = exp(S^T) * ew_dup[jw(j), q],  ew = exp(RW^T)
  PV + row-sums fused: vT_aug = [vT | 1] so out rows 0..63 = unnormalized
  attention output (transposed [d, q]), row 64 = softmax denominators.
  Output projection partial: P = Wout[:, i*64:(i+1)*64] @ out, then columns
  scaled by 1/sums (normalization commutes with the d-contraction).
Host sums the 4 per-head partials per batch and adds the residual x.

All inputs are pre-converted to bf16 on host (no on-device casts). The
steady-state pipeline is ACT(exp)-bound at ~1.1us per 128-key chunk; the
schedule keeps ACT saturated: PE builds/projections go to a dedicated PSUM
slot, psum->sbuf copies go to DVE, softmax denominators use the fast
approximate reciprocal, ew is exp'd in per-quarter chunks so quarter 0 can
start before the full width table is ready.
"""

import numpy as np
import ml_dtypes

import concourse.bass as bass
import concourse.bacc as bacc
import concourse.tile as tile
from concourse import mybir
from concourse.bass_utils import run_bass_kernel_spmd

F32 = mybir.dt.float32
BF16 = mybir.dt.bfloat16
AF = mybir.ActivationFunctionType

HEADS, B, C, HH, WW = 4, 2, 256, 64, 64
N = HH * WW           # 4096
DH = C // HEADS       # 64
NQ = 4                # query blocks
QB = N // NQ          # 1024 query cols per block
NJC = 32              # key chunks of 128
PVLAG = 5


def _body(tc, io):
    from contextlib import ExitStack
    with ExitStack() as ctx:
        _body_inner(tc, io, ctx)


def _body_inner(tc, io, ctx):
    nc = tc.nc
    xb, wq, wk, wv, wo, relw, relh, ih, out = (
        io["xb"], io["wq"], io["wk"], io["wv"], io["wo"],
        io["relw"], io["relh"], io["ih"], io["out"],
    )

    big = ctx.enter_context(tc.tile_pool(name="big", bufs=1))
    rot = ctx.enter_context(tc.tile_pool(name="rot", bufs=16))
    ep = ctx.enter_context(tc.tile_pool(name="ep", bufs=2))
    spool = ctx.enter_context(tc.tile_pool(name="spool", bufs=2, space="PSUM"))
    opool = ctx.enter_context(tc.tile_pool(name="opool", bufs=1, space="PSUM"))
    ipool = ctx.enter_context(tc.tile_pool(name="ipool", bufs=1, space="PSUM"))
    dpool = ctx.enter_context(tc.tile_pool(name="dpool", bufs=2, space="DRAM"))

    # ---- SBUF tiles -------------------------------------------------
    xb_bf = big.tile([128, 2, N], BF16)
    wq_bf = big.tile([128, 2, DH], BF16)
    wk_bf = big.tile([128, 2, DH], BF16)
    wv_bf = big.tile([128, 2, DH], BF16)
    wo_bf = big.tile([64, 256], BF16)
    relw_bf = big.tile([64, 127], BF16)
    relh_bf = big.tile([64, 127], BF16)
    K_aug = big.tile([128, N], BF16)
    Q_aug = big.tile([128, N], BF16)
    ew_dup = big.tile([128, N], BF16)
    rwt = big.tile([64, N], BF16)
    vt_aug = big.tile([128, NJC, 65], BF16)
    h_sb = big.tile([64, N], BF16)

    # ---- input DMAs ------------------------------------------------
    # First-needed data first; weight loads go on the gpsimd queue so the
    # sync queue's issue cost doesn't delay xb quarter 0.
    xv = xb.rearrange("(cc p) n -> p cc n", p=128)
    nc.sync.dma_start(out=xb_bf[:, 0, 0:QB], in_=xv[:, 0, 0:QB])
    nc.gpsimd.dma_start(out=xb_bf[:, 1, 0:QB], in_=xv[:, 1, 0:QB])
    for t_bf, t_d in ((wq_bf, wq), (wk_bf, wk), (wv_bf, wv)):
        nc.gpsimd.dma_start(out=t_bf, in_=t_d.rearrange("(cc p) d -> p cc d", p=128))
    nc.sync.dma_start(out=relh_bf, in_=relh)
    for qq in range(1, NQ):
        nc.sync.dma_start(out=xb_bf[:, 0, bass.ts(qq, QB)], in_=xv[:, 0, bass.ts(qq, QB)])
        nc.gpsimd.dma_start(out=xb_bf[:, 1, bass.ts(qq, QB)], in_=xv[:, 1, bass.ts(qq, QB)])
    nc.sync.dma_start(out=relw_bf, in_=relw)
    # Ih rows of K_aug straight from dram (bf16, exact 0/1)
    nc.sync.dma_start(out=K_aug[64:128, :], in_=ih)
    nc.gpsimd.dma_start(out=wo_bf, in_=wo)

    nc.gpsimd.memset(vt_aug[:, :, 64:65], 1.0)
    ones_row = big.tile([1, 128], BF16)
    nc.gpsimd.memset(ones_row, 1.0)

    # PE warm-up while the first xb quarter lands.
    warm = big.tile([128, 512], BF16)
    nc.vector.memset(warm, 0.0)
    for _ in range(9):
        wps = spool.tile([128, 512], F32, tag="sp")
        nc.tensor.matmul(wps, warm[:, 0:128], warm, start=True, stop=True)

    # ---- build helpers ---------------------------------------------
    # psum->sbuf copies: ACT while it is idle (prologue), DVE in-loop.
    def _copy(eng, dst, src):
        if eng == "act":
            nc.scalar.activation(out=dst, in_=src, func=AF.Copy)
        else:
            nc.vector.tensor_copy(out=dst, in_=src)

    def qk_build(dst, w_bf, qq, pool, tag, eng="dve"):
        ps = pool.tile([128, QB], F32, tag=tag)
        for cc in range(2):
            for h in range(2):
                nc.tensor.matmul(
                    ps[0:64, bass.ts(h, 512)],
                    w_bf[:, cc, :],
                    xb_bf[:, cc, qq * QB + h * 512: qq * QB + (h + 1) * 512],
                    start=(cc == 0), stop=(cc == 1),
                )
        _copy(eng, dst[0:64, bass.ts(qq, QB)], ps[0:64, :])

    def rh_build(g, pool, tag, eng="dve"):
        # RH^T[jh, n=(x,y)] = sum_d relh[jh - x + 63, d] * q[d, n]
        ps = pool.tile([128, QB], F32, tag=tag)
        for xi in range(16):
            xx = g * 16 + xi
            nc.tensor.matmul(
                ps[0:64, bass.ts(xi, 64)],
                relh_bf[:, 63 - xx: 127 - xx],
                Q_aug[0:64, xx * 64: (xx + 1) * 64],
                start=True, stop=True,
            )
        _copy(eng, Q_aug[64:128, bass.ts(g, QB)], ps[0:64, :])

    q_xy = Q_aug[0:64, :].rearrange("d (x y) -> d x y", y=64)
    rwt_xy = rwt.rearrange("jw (x y) -> jw x y", y=64)

    def rw_build(g, pool, tag, eng="dve"):
        # RW^T[jw, n=(x,y)] = sum_d relw[jw - y + 63, d] * q[d, n]
        ps = pool.tile([128, QB], F32, tag=tag)
        for yi in range(16):
            yy = g * 16 + yi
            nc.tensor.matmul(
                ps[0:64, bass.ts(yi, 64)],
                relw_bf[:, 63 - yy: 127 - yy],
                q_xy[:, :, yy],
                start=True, stop=True,
            )
        # ps free layout is [yi, x]; rwt quarter slice wants [x, y].
        _copy(eng, rwt_xy[:, :, g * 16:(g + 1) * 16],
              ps[0:64, :].rearrange("p (yi x) -> p x yi", x=64))

    def vt_build(g, pool, tag, eng="dve"):
        ps = pool.tile([128, 8, 64], F32, tag=tag)
        for ci in range(8):
            chunk = g * 8 + ci
            for cc in range(2):
                nc.tensor.matmul(
                    ps[:, ci, :],
                    xb_bf[:, cc, chunk * 128: (chunk + 1) * 128],
                    wv_bf[:, cc, :],
                    start=(cc == 0), stop=(cc == 1),
                )
        _copy(eng, vt_aug[:, g * 8: (g + 1) * 8, 0:64], ps)

    def ew_build(c):
        # ew chunk c covers query x-block c; only quarter c's muls need it.
        nc.scalar.activation(out=ew_dup[0:64, bass.ts(c, QB)],
                             in_=rwt[:, bass.ts(c, QB)], func=AF.Exp)
        nc.vector.tensor_copy(out=ew_dup[64:128, bass.ts(c, QB)],
                              in_=ew_dup[0:64, bass.ts(c, QB)])

    # ---- prologue --------------------------------------------------
    # Only what gates S(0) and exp(0): Q0/rh0/K0 (+Q1/V0). rw/ew gate only
    # the e-muls and are injected into quarter 0's stream -- the exp stream
    # runs up to ~13 chunks ahead of the first e-mul (deep e0 ring).
    # Copy engines: ACT takes the chain gating its own exp stream, DVE the
    # rest, so the two copy chains run in parallel.
    qk_build(Q_aug, wq_bf, 0, spool, "sp", eng="act")
    qk_build(Q_aug, wq_bf, 1, spool, "sp", eng="act")
    rh_build(0, spool, "sp", eng="act")
    qk_build(K_aug, wk_bf, 0, spool, "sp", eng="dve")
    vt_build(0, spool, "sp", eng="dve")

    # ---- main attention loop ---------------------------------------
    # Per chunk: S^T matmul (PE) -> exp (ACT) -> *ew (DVE) -> PV (PE).
    # ACT is the bottleneck engine; everything else is scheduled around it.
    def make_proj(qqp, rbc):
        def proj(oh, pool=ipool, tag="ij"):
            pp = pool.tile([128, QB], F32, tag=tag)
            for h in range(2):
                nc.tensor.matmul(
                    pp[:, bass.ts(h, 512)],
                    wo_bf[:, oh * 128: (oh + 1) * 128],
                    h_sb[:, qqp * QB + h * 512: qqp * QB + (h + 1) * 512],
                    start=True, stop=True,
                )
            osb = ep.tile([128, QB], F32, tag="osb")
            nc.vector.tensor_mul(osb, pp, rbc)
            eng = nc.sync if oh == 0 else nc.gpsimd
            eng.dma_start(
                out=out[oh * 128: (oh + 1) * 128, qqp * QB: (qqp + 1) * QB],
                in_=osb,
            )
        return proj

    o_ps = None
    proj_prev = None

    def drain(qqp, pool, tag):
        # softmax denominators -> 1/sums, broadcast to 128 partitions with a
        # PE ones-outer-product (no slow dram round-trip); unnormalized
        # attention rows -> h_sb (bf16 cast).
        rs0 = ep.tile([1, QB], F32, tag="rs0")
        nc.vector.tensor_copy(out=rs0, in_=o_ps[64:65, :])
        nc.vector.tensor_copy(out=h_sb[:, bass.ts(qqp, QB)], in_=o_ps[0:64, :])
        rsb = ep.tile([1, QB], F32, tag="rsb")
        nc.vector.reciprocal_approx_fast(out=rsb, in_=rs0)
        rsb_bf = ep.tile([1, QB], BF16, tag="rsbf")
        nc.vector.tensor_copy(out=rsb_bf, in_=rsb)
        rbc_ps = pool.tile([128, QB], F32, tag=tag)
        for h in range(2):
            nc.tensor.matmul(rbc_ps[:, bass.ts(h, 512)], ones_row,
                             rsb_bf[:, bass.ts(h, 512)], start=True, stop=True)
        rbc = ep.tile([128, QB], BF16, tag="rbc")
        nc.vector.tensor_copy(out=rbc, in_=rbc_ps)
        return make_proj(qqp, rbc)

    for qq in range(NQ):
        if qq > 0:
            proj_prev = drain(qq - 1, ipool, "ij")
        o_ps = opool.tile([128, QB], F32, tag="ov")
        e_tiles = [None] * NJC

        e0_tiles = [None] * NJC

        def mul_stage(jc):
            e = rot.tile([128, QB], BF16, tag="e")
            nc.vector.tensor_mul(e, e0_tiles[jc], ew_dup[:, bass.ts(qq, QB)])
            e0_tiles[jc] = None
            e_tiles[jc] = e

        def s_stage(jc, do_mul=True):
            ps = spool.tile([128, QB], F32, tag="sp")
            for h in range(2):
                nc.tensor.matmul(
                    ps[:, bass.ts(h, 512)],
                    K_aug[:, jc * 128: (jc + 1) * 128],
                    Q_aug[:, qq * QB + h * 512: qq * QB + (h + 1) * 512],
                    start=True, stop=True,
                )
            e0 = rot.tile([128, QB], BF16, tag="e0")
            nc.scalar.activation(out=e0, in_=ps, func=AF.Exp)
            e0_tiles[jc] = e0
            if do_mul:
                mul_stage(jc)

        def pv_stage(jc):
            for h in range(2):
                nc.tensor.matmul(
                    o_ps[0:65, bass.ts(h, 512)],
                    vt_aug[:, jc, :],
                    e_tiles[jc][:, bass.ts(h, 512)],
                    start=(jc == 0), stop=(jc == NJC - 1),
                )
            e_tiles[jc] = None

        if qq == 0:
            # Quarter 0: rw/ew injected early (they gate the e-muls; the exp
            # stream runs ahead into the e0 ring). PV emission is deferred to
            # t>=16, two per step, so the PE stream never blocks on the first
            # e-mul.
            inj = {0: lambda: qk_build(Q_aug, wq_bf, 2, ipool, "ij"),
                   2: lambda: qk_build(Q_aug, wq_bf, 3, ipool, "ij"),
                   3: lambda: qk_build(K_aug, wk_bf, 1, ipool, "ij"),
                   4: lambda: rw_build(0, ipool, "ij"),
                   6: lambda: rw_build(1, ipool, "ij"),
                   7: lambda: vt_build(1, ipool, "ij"),
                   8: lambda: rw_build(2, ipool, "ij"),
                   10: lambda: rw_build(3, ipool, "ij"),
                   12: lambda: ew_build(0),
                   13: lambda: qk_build(K_aug, wk_bf, 2, ipool, "ij"),
                   15: lambda: rh_build(1, ipool, "ij"),
                   17: lambda: vt_build(2, ipool, "ij"),
                   21: lambda: qk_build(K_aug, wk_bf, 3, ipool, "ij"),
                   24: lambda: ew_build(1),
                   26: lambda: vt_build(3, ipool, "ij")}
            pv_next = 0
            for t in range(NJC):
                # e-muls read ew_dup, written by ew_build(0) at t==12: defer
                # their emission until after it (emission order = data order).
                s_stage(t, do_mul=(t > 12))
                if t in inj:
                    inj[t]()
                    if t == 12:
                        for jc in range(13):
                            mul_stage(jc)
                if t >= 16:
                    for _ in range(2):
                        if pv_next <= t - 2 and pv_next < NJC:
                            pv_stage(pv_next)
                            pv_next += 1
            while pv_next < NJC:
                pv_stage(pv_next)
                pv_next += 1
        else:
            for t in range(NJC + PVLAG):
                if t < NJC:
                    s_stage(t)
                if t == 6:
                    proj_prev(0)
                elif t == 10:
                    proj_prev(1)
                elif qq == 1:
                    if t == 2:
                        rh_build(2, ipool, "ij")
                    elif t == 24:
                        ew_build(2)
                elif qq == 2:
                    if t == 2:
                        rh_build(3, ipool, "ij")
                    elif t == 24:
                        ew_build(3)
                if t >= PVLAG:
                    pv_stage(t - PVLAG)

    # final quarter epilogue: projections on free S-pool slots so the two
    # output halves run in parallel instead of serializing through ipool.
    proj_last = drain(NQ - 1, spool, "sp")
    proj_last(0, pool=spool, tag="sp")
    proj_last(1, pool=spool, tag="sp")


_NC_CACHE = {}


def _build():
    if "nc" in _NC_CACHE:
        return _NC_CACHE["nc"]
    nc = bacc.Bacc("TRN2", target_bir_lowering=False, debug=False, num_devices=8)
    io = {
        "xb": nc.dram_tensor("xb", [C, N], BF16, kind="ExternalInput").ap(),
        "wq": nc.dram_tensor("wq", [C, DH], BF16, kind="ExternalInput").ap(),
        "wk": nc.dram_tensor("wk", [C, DH], BF16, kind="ExternalInput").ap(),
        "wv": nc.dram_tensor("wv", [C, DH], BF16, kind="ExternalInput").ap(),
        "wo": nc.dram_tensor("wo", [DH, C], BF16, kind="ExternalInput").ap(),
        "relw": nc.dram_tensor("relw", [DH, 127], BF16, kind="ExternalInput").ap(),
        "relh": nc.dram_tensor("relh", [DH, 127], BF16, kind="ExternalInput").ap(),
        "ih": nc.dram_tensor("ih", [64, N], BF16, kind="ExternalInput").ap(),
        "out": nc.dram_tensor("out", [C, N], F32, kind="ExternalOutput").ap(),
    }
    with tile.TileContext(nc) as tc:
        _body(tc, io)
    nc.compile()
    _NC_CACHE["nc"] = nc
    return nc


_last_in_maps = None


def kernel(x, w_qkv, w_out, rel_height, rel_width):
    global _last_in_maps
    bf16 = ml_dtypes.bfloat16
    x = np.ascontiguousarray(np.asarray(x, np.float32))
    w_qkv = np.asarray(w_qkv, np.float32)
    w_out = np.asarray(w_out, np.float32)
    rel_height = np.asarray(rel_height, np.float32)
    rel_width = np.asarray(rel_width, np.float32)

    scale = np.float32(DH ** -0.5)
    ih_const = np.ascontiguousarray(
        np.repeat(np.eye(64, dtype=np.float32), 64, axis=1).astype(bf16))
    relw_t = np.ascontiguousarray(rel_width.T.astype(bf16))
    relh_t = np.ascontiguousarray(rel_height.T.astype(bf16))

    xb_bf = [np.ascontiguousarray(x[b].reshape(C, N).astype(bf16)) for b in range(B)]

    in_maps = []
    for g in range(8):
        b, i = divmod(g, HEADS)
        sl = slice(i * DH, (i + 1) * DH)
        in_maps.append({
            "xb": xb_bf[b],
            "wq": np.ascontiguousarray((w_qkv[i * DH:(i + 1) * DH] * scale).T.astype(bf16)),
            "wk": np.ascontiguousarray(w_qkv[C + i * DH: C + (i + 1) * DH].T.astype(bf16)),
            "wv": np.ascontiguousarray(w_qkv[2 * C + i * DH: 2 * C + (i + 1) * DH].T.astype(bf16)),
            "wo": np.ascontiguousarray(w_out[:, sl].T.astype(bf16)),
            "relw": relw_t,
            "relh": relh_t,
            "ih": ih_const,
        })

    _last_in_maps = in_maps
    nc = _build()
    res = run_bass_kernel_spmd(nc, in_maps, core_ids=list(range(8)))
    parts = [r["out"] for r in res.results]
    outf = np.empty((B, C, N), np.float32)
    for b in range(B):
        outf[b] = parts[4 * b] + parts[4 * b + 1] + parts[4 * b + 2] + parts[4 * b + 3]
        outf[b] += x[b].reshape(C, N)
    return outf.reshape(B, C, HH, WW)


# revision 29
# speedup vs baseline: 1.2218x; 1.2006x over previous
"""BottleneckAttention TRN2 kernel: 8 NeuronCores, one (batch, head) pair per core.

Decomposition (per core, batch b / head i):
  q = (scale * Wq_i) @ x_b          [64, 4096]   (d-major)
  k = Wk_i @ x_b                    [64, 4096]
  vT = (Wv_i @ x_b)^T               [4096, 64]   (n-major, built chunkwise)
  Height rel-bias folded into the score matmul via an augmented contraction:
     K_aug = [k; Ih]  Q_aug = [q; RH^T]     (Ih[h',j] = 1 if j//64 == h')
     S^T[j,q] = K_aug^T Q_aug = content + height-bias
  Width rel-bias applied multiplicatively after exp (separability of exp):
     E = exp(S^T) * ew_dup[jw(j), q],  ew = exp(RW^T)
  PV + row-sums fused: vT_aug = [vT | 1] so out rows 0..63 = unnormalized
  attention output (transposed [d, q]), row 64 = softmax denominators.
  Output projection partial: P = Wout[:, i*64:(i+1)*64] @ out, then columns
  scaled by 1/sums (normalization commutes with the d-contraction).
Host sums the 4 per-head partials per batch and adds the residual x.

All inputs are pre-converted to bf16 on host (no on-device casts). The
steady-state pipeline is ACT(exp)-bound at ~1.1us per 128-key chunk; the
schedule keeps ACT saturated: PE builds/projections go to a dedicated PSUM
slot, psum->sbuf copies go to DVE, softmax denominators use the fast
approximate reciprocal, ew is exp'd in per-quarter chunks so quarter 0 can
start before the full width table is ready.
"""

import numpy as np
import ml_dtypes

import concourse.bass as bass
import concourse.bacc as bacc
import concourse.tile as tile
from concourse import mybir
from concourse.bass_utils import run_bass_kernel_spmd

F32 = mybir.dt.float32
BF16 = mybir.dt.bfloat16
AF = mybir.ActivationFunctionType

HEADS, B, C, HH, WW = 4, 2, 256, 64, 64
N = HH * WW           # 4096
DH = C // HEADS       # 64
NQ = 4                # query blocks
QB = N // NQ          # 1024 query cols per block
NJC = 32              # key chunks of 128
PVLAG = 5


def _body(tc, io):
    from contextlib import ExitStack
    with ExitStack() as ctx:
        _body_inner(tc, io, ctx)


def _body_inner(tc, io, ctx):
    nc = tc.nc
    xb, wq, wk, wv, wo, relw, relh, ih, out = (
        io["xb"], io["wq"], io["wk"], io["wv"], io["wo"],
        io["relw"], io["relh"], io["ih"], io["out"],
    )

    big = ctx.enter_context(tc.tile_pool(name="big", bufs=1))
    rot = ctx.enter_context(tc.tile_pool(name="rot", bufs=16))
    ep = ctx.enter_context(tc.tile_pool(name="ep", bufs=2))
    spool = ctx.enter_context(tc.tile_pool(name="spool", bufs=2, space="PSUM"))
    opool = ctx.enter_context(tc.tile_pool(name="opool", bufs=1, space="PSUM"))
    ipool = ctx.enter_context(tc.tile_pool(name="ipool", bufs=1, space="PSUM"))
    dpool = ctx.enter_context(tc.tile_pool(name="dpool", bufs=2, space="DRAM"))

    # ---- SBUF tiles -------------------------------------------------
    xb_bf = big.tile([128, 2, N], BF16)
    wq_bf = big.tile([128, 2, DH], BF16)
    wk_bf = big.tile([128, 2, DH], BF16)
    wv_bf = big.tile([128, 2, DH], BF16)
    wo_bf = big.tile([64, 256], BF16)
    relw_bf = big.tile([64, 127], BF16)
    relh_bf = big.tile([64, 127], BF16)
    K_aug = big.tile([128, N], BF16)
    Q_aug = big.tile([128, N], BF16)
    ew_dup = big.tile([128, N], BF16)
    rwt = big.tile([64, N], BF16)
    vt_aug = big.tile([128, NJC, 65], BF16)
    h_sb = big.tile([64, N], BF16)

    # ---- input DMAs ------------------------------------------------
    # First-needed data first; weight loads go on the gpsimd queue so the
    # sync queue's issue cost doesn't delay xb quarter 0.
    xv = xb.rearrange("(cc p) n -> p cc n", p=128)
    nc.sync.dma_start(out=xb_bf[:, 0, 0:QB], in_=xv[:, 0, 0:QB])
    nc.gpsimd.dma_start(out=xb_bf[:, 1, 0:QB], in_=xv[:, 1, 0:QB])
    for t_bf, t_d in ((wq_bf, wq), (wk_bf, wk), (wv_bf, wv)):
        nc.gpsimd.dma_start(out=t_bf, in_=t_d.rearrange("(cc p) d -> p cc d", p=128))
    nc.sync.dma_start(out=relh_bf, in_=relh)
    for qq in range(1, NQ):
        nc.sync.dma_start(out=xb_bf[:, 0, bass.ts(qq, QB)], in_=xv[:, 0, bass.ts(qq, QB)])
        nc.gpsimd.dma_start(out=xb_bf[:, 1, bass.ts(qq, QB)], in_=xv[:, 1, bass.ts(qq, QB)])
    nc.sync.dma_start(out=relw_bf, in_=relw)
    # Ih rows of K_aug straight from dram (bf16, exact 0/1)
    nc.sync.dma_start(out=K_aug[64:128, :], in_=ih)
    nc.gpsimd.dma_start(out=wo_bf, in_=wo)

    nc.gpsimd.memset(vt_aug[:, :, 64:65], 1.0)
    ones_row = big.tile([1, 128], BF16)
    nc.gpsimd.memset(ones_row, 1.0)

    # PE warm-up while the first xb quarter lands.
    warm = big.tile([128, 512], BF16)
    nc.vector.memset(warm, 0.0)
    for _ in range(9):
        wps = spool.tile([128, 512], F32, tag="sp")
        nc.tensor.matmul(wps, warm[:, 0:128], warm, start=True, stop=True)

    # ---- build helpers ---------------------------------------------
    # psum->sbuf copies: ACT while it is idle (prologue), DVE in-loop.
    def _copy(eng, dst, src):
        if eng == "act":
            nc.scalar.activation(out=dst, in_=src, func=AF.Copy)
        else:
            nc.vector.tensor_copy(out=dst, in_=src)

    def qk_build(dst, w_bf, qq, pool, tag, eng="dve"):
        ps = pool.tile([128, QB], F32, tag=tag)
        for cc in range(2):
            for h in range(2):
                nc.tensor.matmul(
                    ps[0:64, bass.ts(h, 512)],
                    w_bf[:, cc, :],
                    xb_bf[:, cc, qq * QB + h * 512: qq * QB + (h + 1) * 512],
                    start=(cc == 0), stop=(cc == 1),
                )
        _copy(eng, dst[0:64, bass.ts(qq, QB)], ps[0:64, :])

    def rh_build(g, pool, tag, eng="dve"):
        # RH^T[jh, n=(x,y)] = sum_d relh[jh - x + 63, d] * q[d, n]
        ps = pool.tile([128, QB], F32, tag=tag)
        for xi in range(16):
            xx = g * 16 + xi
            nc.tensor.matmul(
                ps[0:64, bass.ts(xi, 64)],
                relh_bf[:, 63 - xx: 127 - xx],
                Q_aug[0:64, xx * 64: (xx + 1) * 64],
                start=True, stop=True,
            )
        _copy(eng, Q_aug[64:128, bass.ts(g, QB)], ps[0:64, :])

    q_xy = Q_aug[0:64, :].rearrange("d (x y) -> d x y", y=64)
    rwt_xy = rwt.rearrange("jw (x y) -> jw x y", y=64)

    def rw_build(g, pool, tag, eng="dve"):
        # RW^T[jw, n=(x,y)] = sum_d relw[jw - y + 63, d] * q[d, n]
        ps = pool.tile([128, QB], F32, tag=tag)
        for yi in range(16):
            yy = g * 16 + yi
            nc.tensor.matmul(
                ps[0:64, bass.ts(yi, 64)],
                relw_bf[:, 63 - yy: 127 - yy],
                q_xy[:, :, yy],
                start=True, stop=True,
            )
        # ps free layout is [yi, x]; rwt quarter slice wants [x, y].
        _copy(eng, rwt_xy[:, :, g * 16:(g + 1) * 16],
              ps[0:64, :].rearrange("p (yi x) -> p x yi", x=64))

    def vt_build(g, pool, tag, eng="dve"):
        ps = pool.tile([128, 8, 64], F32, tag=tag)
        for ci in range(8):
            chunk = g * 8 + ci
            for cc in range(2):
                nc.tensor.matmul(
                    ps[:, ci, :],
                    xb_bf[:, cc, chunk * 128: (chunk + 1) * 128],
                    wv_bf[:, cc, :],
                    start=(cc == 0), stop=(cc == 1),
                )
        _copy(eng, vt_aug[:, g * 8: (g + 1) * 8, 0:64], ps)

    def ew_build(c):
        # ew chunk c covers query x-block c; only quarter c's muls need it.
        nc.scalar.activation(out=ew_dup[0:64, bass.ts(c, QB)],
                             in_=rwt[:, bass.ts(c, QB)], func=AF.Exp)
        nc.vector.tensor_copy(out=ew_dup[64:128, bass.ts(c, QB)],
                              in_=ew_dup[0:64, bass.ts(c, QB)])

    # ---- prologue: all Q/K/V/rw builds -----------------------------
    # The main loop's quarter 0 has zero PE slack (S + deferred PV fill it),
    # so every build lives here, where ACT/DVE are otherwise idle. Builds
    # round-robin through 4 psum slots (spool x2 + ipool + opool, all free
    # before the loop) so a build never waits on the previous build's copy;
    # copies alternate between ACT and DVE so neither chain lags.
    slots = [(spool, "sp"), (spool, "sp"), (ipool, "ij"), (opool, "ov")]
    engs = ["act", "dve"]
    builds = ([("q", i) for i in range(4)]
              + [("rh", 0), ("k", 0), ("v", 0), ("k", 1), ("v", 1),
                 ("k", 2), ("v", 2), ("k", 3), ("v", 3),
                 ("rw", 0), ("rw", 1), ("rw", 2), ("rw", 3)])
    for i, (kind, idx) in enumerate(builds):
        pool, tag = slots[i % 4]
        eng = engs[i % 2]
        if kind == "q":
            qk_build(Q_aug, wq_bf, idx, pool, tag, eng)
        elif kind == "k":
            qk_build(K_aug, wk_bf, idx, pool, tag, eng)
        elif kind == "v":
            vt_build(idx, pool, tag, eng)
        elif kind == "rh":
            rh_build(idx, pool, tag, eng)
        else:
            rw_build(idx, pool, tag, "dve")
    ew_build(0)

    # ---- main attention loop ---------------------------------------
    # Per chunk: S^T matmul (PE) -> exp (ACT) -> *ew (DVE) -> PV (PE).
    # ACT is the bottleneck engine; everything else is scheduled around it.
    def make_proj(qqp, rbc):
        def proj(oh, pool=ipool, tag="ij"):
            pp = pool.tile([128, QB], F32, tag=tag)
            for h in range(2):
                nc.tensor.matmul(
                    pp[:, bass.ts(h, 512)],
                    wo_bf[:, oh * 128: (oh + 1) * 128],
                    h_sb[:, qqp * QB + h * 512: qqp * QB + (h + 1) * 512],
                    start=True, stop=True,
                )
            osb = ep.tile([128, QB], F32, tag="osb")
            nc.vector.tensor_mul(osb, pp, rbc)
            eng = nc.sync if oh == 0 else nc.gpsimd
            eng.dma_start(
                out=out[oh * 128: (oh + 1) * 128, qqp * QB: (qqp + 1) * QB],
                in_=osb,
            )
        return proj

    o_ps = None
    proj_prev = None

    def drain(qqp, pool, tag):
        # softmax denominators -> 1/sums, broadcast to 128 partitions with a
        # PE ones-outer-product (no slow dram round-trip); unnormalized
        # attention rows -> h_sb (bf16 cast).
        rs0 = ep.tile([1, QB], F32, tag="rs0")
        nc.vector.tensor_copy(out=rs0, in_=o_ps[64:65, :])
        nc.vector.tensor_copy(out=h_sb[:, bass.ts(qqp, QB)], in_=o_ps[0:64, :])
        rsb = ep.tile([1, QB], F32, tag="rsb")
        nc.vector.reciprocal_approx_fast(out=rsb, in_=rs0)
        rsb_bf = ep.tile([1, QB], BF16, tag="rsbf")
        nc.vector.tensor_copy(out=rsb_bf, in_=rsb)
        rbc_ps = pool.tile([128, QB], F32, tag=tag)
        for h in range(2):
            nc.tensor.matmul(rbc_ps[:, bass.ts(h, 512)], ones_row,
                             rsb_bf[:, bass.ts(h, 512)], start=True, stop=True)
        rbc = ep.tile([128, QB], BF16, tag="rbc")
        nc.vector.tensor_copy(out=rbc, in_=rbc_ps)
        return make_proj(qqp, rbc)

    for qq in range(NQ):
        if qq > 0:
            proj_prev = drain(qq - 1, ipool, "ij")
        o_ps = opool.tile([128, QB], F32, tag="ov")
        e_tiles = [None] * NJC

        e0_tiles = [None] * NJC

        def mul_stage(jc):
            e = rot.tile([128, QB], BF16, tag="e")
            nc.vector.tensor_mul(e, e0_tiles[jc], ew_dup[:, bass.ts(qq, QB)])
            e0_tiles[jc] = None
            e_tiles[jc] = e

        def s_stage(jc, do_mul=True):
            ps = spool.tile([128, QB], F32, tag="sp")
            for h in range(2):
                nc.tensor.matmul(
                    ps[:, bass.ts(h, 512)],
                    K_aug[:, jc * 128: (jc + 1) * 128],
                    Q_aug[:, qq * QB + h * 512: qq * QB + (h + 1) * 512],
                    start=True, stop=True,
                )
            e0 = rot.tile([128, QB], BF16, tag="e0")
            nc.scalar.activation(out=e0, in_=ps, func=AF.Exp)
            e0_tiles[jc] = e0
            if do_mul:
                mul_stage(jc)

        def pv_stage(jc):
            for h in range(2):
                nc.tensor.matmul(
                    o_ps[0:65, bass.ts(h, 512)],
                    vt_aug[:, jc, :],
                    e_tiles[jc][:, bass.ts(h, 512)],
                    start=(jc == 0), stop=(jc == NJC - 1),
                )
            e_tiles[jc] = None

        for t in range(NJC + PVLAG):
            if t < NJC:
                s_stage(t)
            if qq > 0:
                if t == 6:
                    proj_prev(0)
                elif t == 10:
                    proj_prev(1)
            if qq < NQ - 1:
                if t == 4:
                    rh_build(qq + 1, ipool, "ij")
                elif t == 24:
                    ew_build(qq + 1)
            if t >= PVLAG:
                pv_stage(t - PVLAG)

    # final quarter epilogue: projections on free S-pool slots so the two
    # output halves run in parallel instead of serializing through ipool.
    proj_last = drain(NQ - 1, spool, "sp")
    proj_last(0, pool=spool, tag="sp")
    proj_last(1, pool=spool, tag="sp")


_NC_CACHE = {}


def _build():
    if "nc" in _NC_CACHE:
        return _NC_CACHE["nc"]
    nc = bacc.Bacc("TRN2", target_bir_lowering=False, debug=False, num_devices=8)
    io = {
        "xb": nc.dram_tensor("xb", [C, N], BF16, kind="ExternalInput").ap(),
        "wq": nc.dram_tensor("wq", [C, DH], BF16, kind="ExternalInput").ap(),
        "wk": nc.dram_tensor("wk", [C, DH], BF16, kind="ExternalInput").ap(),
        "wv": nc.dram_tensor("wv", [C, DH], BF16, kind="ExternalInput").ap(),
        "wo": nc.dram_tensor("wo", [DH, C], BF16, kind="ExternalInput").ap(),
        "relw": nc.dram_tensor("relw", [DH, 127], BF16, kind="ExternalInput").ap(),
        "relh": nc.dram_tensor("relh", [DH, 127], BF16, kind="ExternalInput").ap(),
        "ih": nc.dram_tensor("ih", [64, N], BF16, kind="ExternalInput").ap(),
        "out": nc.dram_tensor("out", [C, N], F32, kind="ExternalOutput").ap(),
    }
    with tile.TileContext(nc) as tc:
        _body(tc, io)
    nc.compile()
    _NC_CACHE["nc"] = nc
    return nc


_last_in_maps = None


def kernel(x, w_qkv, w_out, rel_height, rel_width):
    global _last_in_maps
    bf16 = ml_dtypes.bfloat16
    x = np.ascontiguousarray(np.asarray(x, np.float32))
    w_qkv = np.asarray(w_qkv, np.float32)
    w_out = np.asarray(w_out, np.float32)
    rel_height = np.asarray(rel_height, np.float32)
    rel_width = np.asarray(rel_width, np.float32)

    scale = np.float32(DH ** -0.5)
    ih_const = np.ascontiguousarray(
        np.repeat(np.eye(64, dtype=np.float32), 64, axis=1).astype(bf16))
    relw_t = np.ascontiguousarray(rel_width.T.astype(bf16))
    relh_t = np.ascontiguousarray(rel_height.T.astype(bf16))

    xb_bf = [np.ascontiguousarray(x[b].reshape(C, N).astype(bf16)) for b in range(B)]

    in_maps = []
    for g in range(8):
        b, i = divmod(g, HEADS)
        sl = slice(i * DH, (i + 1) * DH)
        in_maps.append({
            "xb": xb_bf[b],
            "wq": np.ascontiguousarray((w_qkv[i * DH:(i + 1) * DH] * scale).T.astype(bf16)),
            "wk": np.ascontiguousarray(w_qkv[C + i * DH: C + (i + 1) * DH].T.astype(bf16)),
            "wv": np.ascontiguousarray(w_qkv[2 * C + i * DH: 2 * C + (i + 1) * DH].T.astype(bf16)),
            "wo": np.ascontiguousarray(w_out[:, sl].T.astype(bf16)),
            "relw": relw_t,
            "relh": relh_t,
            "ih": ih_const,
        })

    _last_in_maps = in_maps
    nc = _build()
    res = run_bass_kernel_spmd(nc, in_maps, core_ids=list(range(8)))
    parts = [r["out"] for r in res.results]
    outf = np.empty((B, C, N), np.float32)
    for b in range(B):
        outf[b] = parts[4 * b] + parts[4 * b + 1] + parts[4 * b + 2] + parts[4 * b + 3]
        outf[b] += x[b].reshape(C, N)
    return outf.reshape(B, C, HH, WW)
